# revision 57
# baseline (speedup 1.0000x reference)
"""Low-rank attention kernel for Trainium2, distributed over 8 NeuronCores.

Math (per batch b):
    u  = q @ Wu            [N, R]
    vp = k @ Wv            [N, R]
    S  = u @ vp.T / sqrt(R)
    out = softmax(S) @ v   [N, D]

Shapes: B=4, N=4096, D=1024, R=32.

Sharding: data-parallel over batch x row-halves -> 8 shards. Core c handles
batch b = c // 2, rows [h*2048, (h+1)*2048) with h = c % 2. Each core gets its
q-shard and the full k/v for its batch. q/k are fed pre-transposed ([D, n]
layout, f16) so every matmul contraction lands on the partition axis with no
on-device transposes. The whole path runs in f16 (inputs are ~N(0,1); f16
keeps max rel err ~9e-4 on the final output; fp32r matmuls stream at only
1 col per 2 cycles, and fp8 fails the 2e-2 gate - measured 4e-2).

Per-core device kernel:
  1. Projections with Wu/Wv host-replicated 4x to [D, 128]: one K=128
     matmul chain per 512 columns yields uT4/vpT4 = u/vp REPLICATED over
     the four 32-partition groups. The K=128 scores matmul then contracts
     all four replicas (scores come out 4x; the exp scale divides it out).
     Replication keeps every PE op at K=128: a K=32 scores matmul would
     block the AV LDWEIGHTS pull-ahead (row-group switch), costing
     ~200ns/pair - measured.
  2. Main loop over n-chunks of 256 rows, key-PAIRS of 256 keys, software-
     pipelined ACROSS chunks with a 3-pair scores/exp lookahead (fills the
     3 pscore banks and hides the chunk-boundary wait on the DVE
     normalization releasing the acc banks):
       scoresT pair [128, 2, 256] = two K=128 matmuls into one PSUM bank
       expT = Exp(scoresT/sqrt(R)/4) -> f16 [128, 2, 256]  (one ACTIVATE)
       out_acc[n128, d512] += expT_tile.T @ v_tile         (PSUM accum over m)
       sum_acc[n128, 2]    += expT_tile.T @ ones
     out = out_acc * (1 / sum_acc)  (softmax normalization folded at the end)
  All input DMAs are issued up front on one HWDGE ring in critical-path
  order (kT q0 -> wu -> qT h0 -> kT q1 -> v0/v1 -> ...), and chunk 0
  interleaves the remaining projection quarters where their data lands.
"""

import numpy as np

B, N, D, R = 4, 4096, 1024, 32
NLOC = N // 2            # rows per core
RSCALE = float(1.0 / np.sqrt(np.float32(R)))

N_CHUNK = 256            # rows of scores computed per PSUM round
D_HALF = 512             # PSUM bank width in fp32

LAST_RESULT = None       # test.py reads exec_time_ns etc. from here


def _build():
    from concourse import bacc, mybir
    from concourse.tile import TileContext

    f32 = mybir.dt.float32
    f16 = mybir.dt.float16
    EXP = mybir.ActivationFunctionType.Exp
    COPY = mybir.ActivationFunctionType.Copy

    nc = bacc.Bacc("TRN2", target_bir_lowering=False)

    DT = D // 128         # 8 d-tiles
    DT_W = DT * 128
    MQ = N // 1024        # 4 column-quarters of kT
    NCH = NLOC // N_CHUNK  # 8 main-loop chunks
    PAIRS = N // 256      # 16 key-pairs (256 keys each)
    VG = 8                # v row-groups of 512
    VPG = N // VG // 128  # 4 key-tiles per v group

    qT = nc.dram_tensor("qT", [D, NLOC], f16, kind="ExternalInput")
    kT = nc.dram_tensor("kT", [D, N], f16, kind="ExternalInput")
    v = nc.dram_tensor("v", [N, D], f16, kind="ExternalInput")
    # Wu/Wv replicated 4x along the rank dim on the host: [D, 128]. The
    # projections then directly produce u/vp replicated over the four
    # 32-partition groups, so the scores matmul is a uniform K=128
    # contraction (no K=32 row-group switch stalling the AV LDWEIGHTS).
    # Both operands replicated makes scores 4x too big; the exp scale
    # divides that back out exactly. Shipped pre-rearranged as
    # [128, DT*128] so the DMA moves 2KB-per-partition contiguous rows.
    wu = nc.dram_tensor("wu", [128, DT_W], f16, kind="ExternalInput")
    wv = nc.dram_tensor("wv", [128, DT_W], f16, kind="ExternalInput")
    # f16 output, host upcasts to f32: halves output DMA (values are <1 in
    # magnitude; f16 adds ~1.5e-4 abs error vs the 6.8e-3 budget)
    o = nc.dram_tensor("o", [NLOC, D], f16, kind="ExternalOutput")

    with TileContext(nc) as tc:
        with tc.tile_pool(name="singles", bufs=1) as singles, \
             tc.tile_pool(name="kqpool", bufs=16) as kqpool, \
             tc.tile_pool(name="qhpool", bufs=4) as qhpool, \
             tc.tile_pool(name="vpool", bufs=VG) as vpool, \
             tc.tile_pool(name="expp", bufs=6) as expp, \
             tc.tile_pool(name="outp", bufs=2) as outp, \
             tc.tile_pool(name="rpool", bufs=4) as rpool, \
             tc.tile_pool(name="pacc", bufs=4, space="PSUM") as pacc, \
             tc.tile_pool(name="pscore", bufs=3, space="PSUM") as pscore, \
             tc.tile_pool(name="psums", bufs=1, space="PSUM") as psums:

            # ---- constants / projection weights ----
            # wv first: the first projection (vp quarter 0) needs it; wu is
            # issued after kT q0 below (only needed once u chunk 0 runs).
            wv_sb = singles.tile([128, DT, 128], f16, tag="wv")
            nc.sync.dma_start(out=wv_sb, in_=wv.rearrange("p (t r) -> p t r", t=DT))
            wu_sb = singles.tile([128, DT, 128], f16, tag="wu")
            ones = singles.tile([128, 2], f16, tag="ones")
            nc.vector.memset(ones, 1.0)

            # u/vp replicated over the four 32-partition groups; the K=128
            # scores matmul then contracts all 4 replicas -> scores carry a
            # 4x factor that the exp scale divides back out.
            uT4 = singles.tile([128, NLOC], f16, tag="uT4")
            vpT4 = singles.tile([128, N], f16, tag="vpT4")

            # ---- all input DMAs issued up front on ONE HWDGE ring (strict
            # FIFO = strict arrival order), in critical-path order: kT q0
            # gates the first projection, qT h0 gates uT chunk 0, kT q1-3 and
            # the v groups arrive just ahead of the chunk-0 pairs that read
            # them. Tiles cover 2 d-tiles per dma_start: half the descriptor
            # issue cost of per-d-tile transfers at 2x the completion grain.
            kq = {}            # (qtr, tp) -> [128, 2, 1024] tile
            qh = {}            # (h, tp) -> [128, 2, 1024] tile
            v_sb = [None] * VG

            def load_kq(qtr):
                for tp in range(DT // 2):
                    tile = kqpool.tile([128, 2, 1024], f16, tag="kq",
                                       name=f"kq{qtr}_{tp}")
                    nc.sync.dma_start(
                        out=tile,
                        in_=kT[tp * 256:(tp + 1) * 256,
                               qtr * 1024:(qtr + 1) * 1024].rearrange(
                            "(t p) c -> p t c", p=128))
                    kq[(qtr, tp)] = tile

            def load_qh(h):
                for tp in range(DT // 2):
                    tile = qhpool.tile([128, 2, 1024], f16, tag="qh",
                                       name=f"qt{h}_{tp}")
                    nc.sync.dma_start(
                        out=tile,
                        in_=qT[tp * 256:(tp + 1) * 256,
                               h * 1024:(h + 1) * 1024].rearrange(
                            "(t p) c -> p t c", p=128))
                    qh[(h, tp)] = tile

            def load_v(g):
                vt = vpool.tile([128, VPG, D], f16, tag="v", name=f"v{g}")
                nc.sync.dma_start(
                    out=vt, in_=v[g * 512:(g + 1) * 512, :].rearrange(
                        "(t p) d -> p t d", p=128))
                v_sb[g] = vt

            load_kq(0)
            nc.sync.dma_start(out=wu_sb, in_=wu.rearrange("p (t r) -> p t r", t=DT))
            load_qh(0)
            load_kq(1)
            load_v(0)
            load_v(1)
            load_kq(2)
            load_v(2)
            load_v(3)
            load_kq(3)
            load_v(4)
            load_v(5)
            load_v(6)
            load_v(7)
            load_qh(1)

            # ---- projection helpers ----
            def u_chunk(c):
                h, off = c // 2, (c % 2) * 512
                pu = pscore.tile([128, 2, 256], f32, tag="scores", name=f"pu{c}")
                for t in range(DT):
                    nc.tensor.matmul(pu, lhsT=wu_sb[:, t, :],
                                     rhs=qh[(h, t // 2)][:, t % 2, off:off + 512],
                                     start=(t == 0), stop=(t == DT - 1))
                for s in range(2):
                    nc.vector.tensor_copy(
                        out=uT4[:, c * 512 + s * 256:c * 512 + (s + 1) * 256],
                        in_=pu[:, s, :])

            def vp_quarter(qtr):
                for c2 in range(2):
                    pv = pscore.tile([128, 2, 256], f32, tag="scores")
                    for t in range(DT):
                        nc.tensor.matmul(pv, lhsT=wv_sb[:, t, :],
                                         rhs=kq[(qtr, t // 2)][:, t % 2,
                                                               c2 * 512:c2 * 512 + 512],
                                         start=(t == 0), stop=(t == DT - 1))
                    off = qtr * 1024 + c2 * 512
                    for s in range(2):
                        nc.vector.tensor_copy(
                            out=vpT4[:, off + s * 256:off + (s + 1) * 256],
                            in_=pv[:, s, :])

            # ---- phase 2: flash-style scores/softmax/AV ----
            # software-pipelined ACROSS chunks: scores/exp for pair i+2 are
            # issued before the AV matmuls of pair i (global pair index), so
            # ScalarE exp latency hides under the previous pair's AV work and
            # chunk boundaries don't drain the pipeline. hooks[(ch, pr)] lets
            # chunk 0 interleave the remaining projection quarters.
            hooks = {
                (0, 4): lambda: vp_quarter(2),
                (0, 8): lambda: vp_quarter(3),
                (0, 12): lambda: (u_chunk(2), u_chunk(3)),
            }
            all_pairs = [(ch, pr) for ch in range(NCH) for pr in range(PAIRS)]

            def scores_exp(ch, pr):
                ps = pscore.tile([128, 2, N_CHUNK], f32, tag="scores",
                                 name=f"ps{ch}_{pr}")
                for s in range(2):
                    m = 2 * pr + s
                    nc.tensor.matmul(
                        ps[:, s, :], lhsT=vpT4[:, m * 128:(m + 1) * 128],
                        rhs=uT4[:, ch * N_CHUNK:(ch + 1) * N_CHUNK],
                        start=True, stop=True, skip_group_check=True)
                ex = expp.tile([128, 2, N_CHUNK], f16, tag="ex",
                               name=f"ex{ch}_{pr}")
                # scores carry a 4x factor from the replicated projections
                nc.scalar.activation(out=ex, in_=ps, func=EXP,
                                     scale=RSCALE / 4.0)
                return ex

            # PE issue order prologue: vp q0 first (kT q0 is the first DMA to
            # land), then u chunk 0/1 (qT h0), vp q1; vp q2/q3 and u chunk 2/3
            # interleave into chunk 0 via hooks once their data has arrived.
            vp_quarter(0)
            u_chunk(0)
            u_chunk(1)
            vp_quarter(1)

            # lookahead 3 pairs: exactly fills pscore's 3 banks, and gives the
            # PE ~6 scores matmuls of slack to hide the chunk-boundary wait on
            # the DVE normalization releasing the acc banks. (Tried pscore=2 +
            # psums=2 instead: boundary stalls got worse and the projection
            # hooks contended for score banks - measured.)
            ex_q = [scores_exp(0, 0), scores_exp(0, 1), scores_exp(0, 2)]
            accs = sums = None
            for i, (ch, pr) in enumerate(all_pairs):
                if (ch, pr) in hooks:
                    hooks[(ch, pr)]()
                if pr == 0:
                    accs = [pacc.tile([128, D_HALF], f32, tag="acc",
                                      name=f"acc{ch}_{k}") for k in range(4)]
                    # both sums accumulators share one bank: start=True clears
                    # has_written bank-wide, so ONLY sums[0]'s first matmul has
                    # start=True; the cleared has_written makes sums[1]'s first
                    # start=False matmul overwrite rather than accumulate
                    sums_t = psums.tile([128, 4], f32, tag="sums",
                                        name=f"sum{ch}")
                    sums = [sums_t[:, 0:2], sums_t[:, 2:4]]
                ex = ex_q.pop(0)
                if i + 3 < len(all_pairs):
                    ex_q.append(scores_exp(*all_pairs[i + 3]))
                g, tg = pr // 2, (pr % 2) * 2
                for s in range(2):
                    first = (pr == 0 and s == 0)
                    last = (pr == PAIRS - 1 and s == 1)
                    for j in range(2):
                        lhs = ex[:, s, j * 128:(j + 1) * 128]
                        nc.tensor.matmul(accs[2 * j], lhsT=lhs,
                                         rhs=v_sb[g][:, tg + s, 0:D_HALF],
                                         start=first, stop=last,
                                         skip_group_check=True)
                        nc.tensor.matmul(accs[2 * j + 1], lhsT=lhs,
                                         rhs=v_sb[g][:, tg + s, D_HALF:D],
                                         start=first, stop=last,
                                         skip_group_check=True)
                        nc.tensor.matmul(sums[j], lhsT=lhs, rhs=ones,
                                         start=(first and j == 0), stop=last,
                                         skip_group_check=True)
                if pr == PAIRS - 1:
                    # normalize on DVE (keeps ScalarE free for the exps); for
                    # the final chunk ScalarE is idle, so its j=1 runs there
                    # in parallel with DVE's j=0 to shorten the tail.
                    for j in range(2):
                        rc = rpool.tile([128, 1], f32, tag="rc",
                                        name=f"rc{ch}_{j}")
                        nc.vector.reciprocal(rc, sums[j][:, 0:1])
                        ob = outp.tile([128, D], f16, tag="ob",
                                       name=f"ob{ch}_{j}")
                        if ch == NCH - 1 and j == 1:
                            nc.scalar.activation(ob[:, 0:D_HALF], accs[2 * j],
                                                 COPY, scale=rc)
                            nc.scalar.activation(ob[:, D_HALF:D],
                                                 accs[2 * j + 1],
                                                 COPY, scale=rc)
                        else:
                            nc.vector.tensor_scalar_mul(ob[:, 0:D_HALF],
                                                        accs[2 * j], rc)
                            nc.vector.tensor_scalar_mul(ob[:, D_HALF:D],
                                                        accs[2 * j + 1], rc)
                        row = ch * N_CHUNK + j * 128
                        nc.sync.dma_start(out=o[row:row + 128, :], in_=ob)

    nc.finalize()
    return nc


def kernel(q, k, v, Wu, Wv):
    global LAST_RESULT
    from concourse import bass_utils

    nc = _build()

    kTs = [np.ascontiguousarray(k[b].T.astype(np.float16)) for b in range(B)]
    vs = [np.ascontiguousarray(v[b]).astype(np.float16) for b in range(B)]
    def prep_w(W):
        w4 = np.tile(W.astype(np.float16), (1, 4))           # [D, 128]
        return np.ascontiguousarray(
            w4.reshape(D // 128, 128, 128).transpose(1, 0, 2).reshape(128, -1))

    wu16 = prep_w(Wu)
    wv16 = prep_w(Wv)
    in_maps = []
    for core in range(8):
        b, h = core // 2, core % 2
        in_maps.append({
            "qT": np.ascontiguousarray(
                q[b].T[:, h * NLOC:(h + 1) * NLOC].astype(np.float16)),
            "kT": kTs[b],
            "v": vs[b],
            "wu": wu16,
            "wv": wv16,
        })

    res = bass_utils.run_bass_kernel_spmd(nc, in_maps, core_ids=list(range(8)))
    LAST_RESULT = res

    out = np.empty((B, N, D), dtype=np.float32)
    for core in range(8):
        b, h = core // 2, core % 2
        out[b, h * NLOC:(h + 1) * NLOC, :] = res.results[core]["o"]  # f16 -> f32
    return out


# revision 59
# speedup vs baseline: 1.0094x; 1.0094x over previous
"""Low-rank attention kernel for Trainium2, distributed over 8 NeuronCores.

Math (per batch b):
    u  = q @ Wu            [N, R]
    vp = k @ Wv            [N, R]
    S  = u @ vp.T / sqrt(R)
    out = softmax(S) @ v   [N, D]

Shapes: B=4, N=4096, D=1024, R=32.

Sharding: data-parallel over batch x row-halves -> 8 shards. Core c handles
batch b = c // 2, rows [h*2048, (h+1)*2048) with h = c % 2. Each core gets its
q-shard and the full k/v for its batch. q/k are fed pre-transposed ([D, n]
layout, f16) so every matmul contraction lands on the partition axis with no
on-device transposes. The whole path runs in f16 (inputs are ~N(0,1); f16
keeps max rel err ~9e-4 on the final output; fp32r matmuls stream at only
1 col per 2 cycles, and fp8 fails the 2e-2 gate - measured 4e-2).

Per-core device kernel:
  1. Projections with Wu/Wv host-replicated 4x to [D, 128]: one K=128
     matmul chain per 512 columns yields uT4/vpT4 = u/vp REPLICATED over
     the four 32-partition groups. The K=128 scores matmul then contracts
     all four replicas (scores come out 4x; the exp scale divides it out).
     Replication keeps every PE op at K=128: a K=32 scores matmul would
     block the AV LDWEIGHTS pull-ahead (row-group switch), costing
     ~200ns/pair - measured.
  2. Main loop over n-chunks of 256 rows, key-PAIRS of 256 keys, software-
     pipelined ACROSS chunks with a 3-pair scores/exp lookahead (fills the
     3 pscore banks and hides the chunk-boundary wait on the DVE
     normalization releasing the acc banks):
       scoresT pair [128, 2, 256] = two K=128 matmuls into one PSUM bank
       expT = Exp(scoresT/sqrt(R)/4) -> f16 [128, 2, 256]  (one ACTIVATE)
       out_acc[n128, d512] += expT_tile.T @ v_tile         (PSUM accum over m)
       sum_acc[n128, 2]    += expT_tile.T @ ones
     out = out_acc * (1 / sum_acc)  (softmax normalization folded at the end)
  All input DMAs are issued up front on one HWDGE ring in critical-path
  order (kT q0 -> wu -> qT h0 -> kT q1 -> v0/v1 -> ...), and chunk 0
  interleaves the remaining projection quarters where their data lands.
"""

import numpy as np

B, N, D, R = 4, 4096, 1024, 32
NLOC = N // 2            # rows per core
RSCALE = float(1.0 / np.sqrt(np.float32(R)))

N_CHUNK = 256            # rows of scores computed per PSUM round
D_HALF = 512             # PSUM bank width in fp32

LAST_RESULT = None       # test.py reads exec_time_ns etc. from here


def _build():
    from concourse import bacc, mybir
    from concourse.tile import TileContext

    f32 = mybir.dt.float32
    f16 = mybir.dt.float16
    EXP = mybir.ActivationFunctionType.Exp
    COPY = mybir.ActivationFunctionType.Copy

    nc = bacc.Bacc("TRN2", target_bir_lowering=False)

    DT = D // 128         # 8 d-tiles
    DT_W = DT * 128
    MQ = N // 1024        # 4 column-quarters of kT
    NCH = NLOC // N_CHUNK  # 8 main-loop chunks
    PAIRS = N // 256      # 16 key-pairs (256 keys each)
    VG = 8                # v row-groups of 512
    VPG = N // VG // 128  # 4 key-tiles per v group

    qT = nc.dram_tensor("qT", [D, NLOC], f16, kind="ExternalInput")
    kT = nc.dram_tensor("kT", [D, N], f16, kind="ExternalInput")
    v = nc.dram_tensor("v", [N, D], f16, kind="ExternalInput")
    # Wu/Wv replicated 4x along the rank dim on the host: [D, 128]. The
    # projections then directly produce u/vp replicated over the four
    # 32-partition groups, so the scores matmul is a uniform K=128
    # contraction (no K=32 row-group switch stalling the AV LDWEIGHTS).
    # Both operands replicated makes scores 4x too big; the exp scale
    # divides that back out exactly. Shipped pre-rearranged as
    # [128, DT*128] so the DMA moves 2KB-per-partition contiguous rows.
    wu = nc.dram_tensor("wu", [128, DT_W], f16, kind="ExternalInput")
    wv = nc.dram_tensor("wv", [128, DT_W], f16, kind="ExternalInput")
    # f16 output, host upcasts to f32: halves output DMA (values are <1 in
    # magnitude; f16 adds ~1.5e-4 abs error vs the 6.8e-3 budget)
    o = nc.dram_tensor("o", [NLOC, D], f16, kind="ExternalOutput")

    with TileContext(nc) as tc:
        with tc.tile_pool(name="singles", bufs=1) as singles, \
             tc.tile_pool(name="kqpool", bufs=16) as kqpool, \
             tc.tile_pool(name="qhpool", bufs=4) as qhpool, \
             tc.tile_pool(name="vpool", bufs=VG) as vpool, \
             tc.tile_pool(name="expp", bufs=6) as expp, \
             tc.tile_pool(name="outp", bufs=2) as outp, \
             tc.tile_pool(name="rpool", bufs=4) as rpool, \
             tc.tile_pool(name="pacc", bufs=4, space="PSUM") as pacc, \
             tc.tile_pool(name="pscore", bufs=3, space="PSUM") as pscore, \
             tc.tile_pool(name="psums", bufs=1, space="PSUM") as psums:

            # ---- HAM warm-up ----
            # The PE sits idle ~3.5-10.7us waiting for the first kT tile, so
            # the projections would start on the cold 1.2GHz clock (HAM needs
            # ~3.4us of sustained matmul activity to unthrottle). These
            # dependency-free dummy matmuls on an uninitialized SBUF tile run
            # right after the engine preamble and finish before real work
            # arrives; their garbage PSUM writes land in an acc-pool slot that
            # chunk 0 later clears with start=True.
            junk = singles.tile([128, 512], f16, tag="junk")
            nc.vector.memset(junk, 1.0)
            warm = pacc.tile([128, D_HALF], f32, tag="acc", name="warm")
            for _ in range(14):
                nc.tensor.matmul(warm, lhsT=junk[:, 0:128], rhs=junk,
                                 start=True, stop=True, skip_group_check=True)

            # ---- constants / projection weights ----
            # wv first: the first projection (vp quarter 0) needs it; wu is
            # issued after kT q0 below (only needed once u chunk 0 runs).
            wv_sb = singles.tile([128, DT, 128], f16, tag="wv")
            nc.sync.dma_start(out=wv_sb, in_=wv.rearrange("p (t r) -> p t r", t=DT))
            wu_sb = singles.tile([128, DT, 128], f16, tag="wu")
            ones = singles.tile([128, 2], f16, tag="ones")
            nc.vector.memset(ones, 1.0)

            # u/vp replicated over the four 32-partition groups; the K=128
            # scores matmul then contracts all 4 replicas -> scores carry a
            # 4x factor that the exp scale divides back out.
            uT4 = singles.tile([128, NLOC], f16, tag="uT4")
            vpT4 = singles.tile([128, N], f16, tag="vpT4")

            # ---- all input DMAs issued up front on ONE HWDGE ring (strict
            # FIFO = strict arrival order), in critical-path order: kT q0
            # gates the first projection, qT h0 gates uT chunk 0, kT q1-3 and
            # the v groups arrive just ahead of the chunk-0 pairs that read
            # them. Tiles cover 2 d-tiles per dma_start: half the descriptor
            # issue cost of per-d-tile transfers at 2x the completion grain.
            kq = {}            # (qtr, tp) -> [128, 2, 1024] tile
            qh = {}            # (h, tp) -> [128, 2, 1024] tile
            v_sb = [None] * VG

            def load_kq(qtr):
                for tp in range(DT // 2):
                    tile = kqpool.tile([128, 2, 1024], f16, tag="kq",
                                       name=f"kq{qtr}_{tp}")
                    nc.sync.dma_start(
                        out=tile,
                        in_=kT[tp * 256:(tp + 1) * 256,
                               qtr * 1024:(qtr + 1) * 1024].rearrange(
                            "(t p) c -> p t c", p=128))
                    kq[(qtr, tp)] = tile

            def load_qh(h):
                for tp in range(DT // 2):
                    tile = qhpool.tile([128, 2, 1024], f16, tag="qh",
                                       name=f"qt{h}_{tp}")
                    nc.sync.dma_start(
                        out=tile,
                        in_=qT[tp * 256:(tp + 1) * 256,
                               h * 1024:(h + 1) * 1024].rearrange(
                            "(t p) c -> p t c", p=128))
                    qh[(h, tp)] = tile

            def load_v(g):
                vt = vpool.tile([128, VPG, D], f16, tag="v", name=f"v{g}")
                nc.sync.dma_start(
                    out=vt, in_=v[g * 512:(g + 1) * 512, :].rearrange(
                        "(t p) d -> p t d", p=128))
                v_sb[g] = vt

            load_kq(0)
            nc.sync.dma_start(out=wu_sb, in_=wu.rearrange("p (t r) -> p t r", t=DT))
            load_qh(0)
            load_kq(1)
            load_v(0)
            load_v(1)
            load_kq(2)
            load_v(2)
            load_v(3)
            load_kq(3)
            load_v(4)
            load_v(5)
            load_v(6)
            load_v(7)
            load_qh(1)

            # ---- projection helpers ----
            def u_chunk(c):
                h, off = c // 2, (c % 2) * 512
                pu = pscore.tile([128, 2, 256], f32, tag="scores", name=f"pu{c}")
                for t in range(DT):
                    nc.tensor.matmul(pu, lhsT=wu_sb[:, t, :],
                                     rhs=qh[(h, t // 2)][:, t % 2, off:off + 512],
                                     start=(t == 0), stop=(t == DT - 1))
                for s in range(2):
                    nc.vector.tensor_copy(
                        out=uT4[:, c * 512 + s * 256:c * 512 + (s + 1) * 256],
                        in_=pu[:, s, :])

            def vp_quarter(qtr):
                for c2 in range(2):
                    pv = pscore.tile([128, 2, 256], f32, tag="scores")
                    for t in range(DT):
                        nc.tensor.matmul(pv, lhsT=wv_sb[:, t, :],
                                         rhs=kq[(qtr, t // 2)][:, t % 2,
                                                               c2 * 512:c2 * 512 + 512],
                                         start=(t == 0), stop=(t == DT - 1))
                    off = qtr * 1024 + c2 * 512
                    for s in range(2):
                        nc.vector.tensor_copy(
                            out=vpT4[:, off + s * 256:off + (s + 1) * 256],
                            in_=pv[:, s, :])

            # ---- phase 2: flash-style scores/softmax/AV ----
            # software-pipelined ACROSS chunks: scores/exp for pair i+2 are
            # issued before the AV matmuls of pair i (global pair index), so
            # ScalarE exp latency hides under the previous pair's AV work and
            # chunk boundaries don't drain the pipeline. hooks[(ch, pr)] lets
            # chunk 0 interleave the remaining projection quarters.
            hooks = {
                (0, 4): lambda: vp_quarter(2),
                (0, 8): lambda: vp_quarter(3),
                (0, 12): lambda: (u_chunk(2), u_chunk(3)),
            }
            all_pairs = [(ch, pr) for ch in range(NCH) for pr in range(PAIRS)]

            def scores_exp(ch, pr):
                ps = pscore.tile([128, 2, N_CHUNK], f32, tag="scores",
                                 name=f"ps{ch}_{pr}")
                for s in range(2):
                    m = 2 * pr + s
                    nc.tensor.matmul(
                        ps[:, s, :], lhsT=vpT4[:, m * 128:(m + 1) * 128],
                        rhs=uT4[:, ch * N_CHUNK:(ch + 1) * N_CHUNK],
                        start=True, stop=True, skip_group_check=True)
                ex = expp.tile([128, 2, N_CHUNK], f16, tag="ex",
                               name=f"ex{ch}_{pr}")
                # scores carry a 4x factor from the replicated projections
                nc.scalar.activation(out=ex, in_=ps, func=EXP,
                                     scale=RSCALE / 4.0)
                return ex

            # PE issue order prologue: vp q0 first (kT q0 is the first DMA to
            # land), then u chunk 0/1 (qT h0), vp q1; vp q2/q3 and u chunk 2/3
            # interleave into chunk 0 via hooks once their data has arrived.
            vp_quarter(0)
            u_chunk(0)
            u_chunk(1)
            vp_quarter(1)

            # lookahead 3 pairs: exactly fills pscore's 3 banks, and gives the
            # PE ~6 scores matmuls of slack to hide the chunk-boundary wait on
            # the DVE normalization releasing the acc banks. (Tried pscore=2 +
            # psums=2 instead: boundary stalls got worse and the projection
            # hooks contended for score banks - measured.)
            ex_q = [scores_exp(0, 0), scores_exp(0, 1), scores_exp(0, 2)]
            accs = sums = None
            for i, (ch, pr) in enumerate(all_pairs):
                if (ch, pr) in hooks:
                    hooks[(ch, pr)]()
                if pr == 0:
                    accs = [pacc.tile([128, D_HALF], f32, tag="acc",
                                      name=f"acc{ch}_{k}") for k in range(4)]
                    # both sums accumulators share one bank: start=True clears
                    # has_written bank-wide, so ONLY sums[0]'s first matmul has
                    # start=True; the cleared has_written makes sums[1]'s first
                    # start=False matmul overwrite rather than accumulate
                    sums_t = psums.tile([128, 4], f32, tag="sums",
                                        name=f"sum{ch}")
                    sums = [sums_t[:, 0:2], sums_t[:, 2:4]]
                ex = ex_q.pop(0)
                if i + 3 < len(all_pairs):
                    ex_q.append(scores_exp(*all_pairs[i + 3]))
                g, tg = pr // 2, (pr % 2) * 2
                for s in range(2):
                    first = (pr == 0 and s == 0)
                    last = (pr == PAIRS - 1 and s == 1)
                    for j in range(2):
                        lhs = ex[:, s, j * 128:(j + 1) * 128]
                        nc.tensor.matmul(accs[2 * j], lhsT=lhs,
                                         rhs=v_sb[g][:, tg + s, 0:D_HALF],
                                         start=first, stop=last,
                                         skip_group_check=True)
                        nc.tensor.matmul(accs[2 * j + 1], lhsT=lhs,
                                         rhs=v_sb[g][:, tg + s, D_HALF:D],
                                         start=first, stop=last,
                                         skip_group_check=True)
                        nc.tensor.matmul(sums[j], lhsT=lhs, rhs=ones,
                                         start=(first and j == 0), stop=last,
                                         skip_group_check=True)
                if pr == PAIRS - 1:
                    # normalize on DVE (keeps ScalarE free for the exps); for
                    # the final chunk ScalarE is idle, so its j=1 runs there
                    # in parallel with DVE's j=0 to shorten the tail.
                    for j in range(2):
                        rc = rpool.tile([128, 1], f32, tag="rc",
                                        name=f"rc{ch}_{j}")
                        nc.vector.reciprocal(rc, sums[j][:, 0:1])
                        ob = outp.tile([128, D], f16, tag="ob",
                                       name=f"ob{ch}_{j}")
                        if ch == NCH - 1 and j == 1:
                            nc.scalar.activation(ob[:, 0:D_HALF], accs[2 * j],
                                                 COPY, scale=rc)
                            nc.scalar.activation(ob[:, D_HALF:D],
                                                 accs[2 * j + 1],
                                                 COPY, scale=rc)
                        else:
                            nc.vector.tensor_scalar_mul(ob[:, 0:D_HALF],
                                                        accs[2 * j], rc)
                            nc.vector.tensor_scalar_mul(ob[:, D_HALF:D],
                                                        accs[2 * j + 1], rc)
                        row = ch * N_CHUNK + j * 128
                        nc.sync.dma_start(out=o[row:row + 128, :], in_=ob)

    nc.finalize()
    return nc


def kernel(q, k, v, Wu, Wv):
    global LAST_RESULT
    from concourse import bass_utils

    nc = _build()

    kTs = [np.ascontiguousarray(k[b].T.astype(np.float16)) for b in range(B)]
    vs = [np.ascontiguousarray(v[b]).astype(np.float16) for b in range(B)]
    def prep_w(W):
        w4 = np.tile(W.astype(np.float16), (1, 4))           # [D, 128]
        return np.ascontiguousarray(
            w4.reshape(D // 128, 128, 128).transpose(1, 0, 2).reshape(128, -1))

    wu16 = prep_w(Wu)
    wv16 = prep_w(Wv)
    in_maps = []
    for core in range(8):
        b, h = core // 2, core % 2
        in_maps.append({
            "qT": np.ascontiguousarray(
                q[b].T[:, h * NLOC:(h + 1) * NLOC].astype(np.float16)),
            "kT": kTs[b],
            "v": vs[b],
            "wu": wu16,
            "wv": wv16,
        })

    res = bass_utils.run_bass_kernel_spmd(nc, in_maps, core_ids=list(range(8)))
    LAST_RESULT = res

    out = np.empty((B, N, D), dtype=np.float32)
    for core in range(8):
        b, h = core // 2, core % 2
        out[b, h * NLOC:(h + 1) * NLOC, :] = res.results[core]["o"]  # f16 -> f32
    return out
